# revision 18
# baseline (speedup 1.0000x reference)
"""Trainium2 Bass kernel for CachingMultiHeadAttention (GQA + RoPE + softcap).

Reference semantics (B=2, S=2048, D=4096, 32 q-heads, 8 kv-heads, hd=128):
    qh = rope(x_q @ Wq); kh = rope(x_k @ Wk); vh = x_v @ Wv
    logits = softcap_30(qh kh^T / sqrt(128)) causal-masked
    out = softmax(logits) vh @ Wo ; also returns cache_k, cache_v

Sharding: 8 cores = (2 batches) x (4 kv-head pairs). Each core computes its
batch's projections for its 2 kv-heads / 8 q-heads against full D_MODEL,
attention, and a partial out^T (summed on host over the 4 cores per batch).

All device matmuls run in float32r (fast fp32 mode, 1 cyc/row at N>=256).
Activations are fed pre-transposed and pre-tiled from the host so every
matmul contracts over the partition dim and every DMA is a large mostly-
contiguous transfer. DMA traffic is spread over three queues (sync/scalar
HWDGE + gpsimd SWDGE). Walrus ldw-opt is enabled (see _patch_ldw_opt) so
consecutive matmuls sharing a stationary skip redundant LDWEIGHTS.

Pipeline:
  A0 wq prefetch (one 16.8MB DMA issued first; wk/wv are streamed per-chunk)
  A1 K proj (+RoPE) -> kht_sb resident      [2 waves, ping-pong PSUM]
  A2 V proj (transposed) + PE-transpose -> vh_sb resident [2 waves]
  A3 Q proj (+RoPE) -> qt_scr[span] DRAM
  B  attention per span -> ot_sb (SBUF)     [softcap=ACT tanh/exp pairs,
                                             additive -100 mask pre-exp,
                                             rowsum = ones-matmul over
                                             DVE+GPSIMD-accumulated P,
                                             approx-recip + partition_bcast]
  C  out proj, 2 half phases (spans 01/23), interleaved with B
"""
import sys
sys.path.insert(0, "/opt/trn_rl_repo")

import numpy as np

import concourse.bass as bass
from concourse import bacc
import concourse.mybir as mybir
import concourse.tile as tile
import concourse.alu_op_type as alu
import concourse.bass_utils as bass_utils
from concourse.bass_utils import run_bass_kernel_spmd

F32 = mybir.dt.float32
F32R = mybir.dt.float32r
AF = mybir.ActivationFunctionType
MUL = alu.AluOpType.mult
ADD = alu.AluOpType.add

# Problem constants
B, S, DM = 2, 2048, 4096
NQ, NKV, HD = 32, 8, 128
GROUP = NQ // NKV
ATTN_MULT = 1.0 / np.sqrt(128.0)
MAX_ATTN = 30.0
ROPE_BASE = 10000.0
MASK_NEG = -100.0            # additive pre-exp mask value (exp(30*-100) == 0)

P = 128
NCORES = 8
HEADS_PER_CORE = NQ // (NCORES // B)      # 8
KV_PER_CORE = NKV // (NCORES // B)        # 2
DQ = HEADS_PER_CORE * HD                  # 1024
DKV = KV_PER_CORE * HD                    # 256

_LDW_PATCHED = False


def _patch_ldw_opt():
    """Enable walrus's LDWEIGHTS dedup pass (hardcoded off in bass_utils)."""
    global _LDW_PATCHED
    if _LDW_PATCHED:
        return
    _LDW_PATCHED = True
    orig = bass_utils.run_command

    def run_command_ldwopt(argv, **kw):
        argv = ['--enable-ldw-opt=true' if a == '--enable-ldw-opt=false'
                else a for a in argv]
        return orig(argv, **kw)

    bass_utils.run_command = run_command_ldwopt


def _rope_tables(s):
    """cos table and sign-baked sin table, [HD, s] f32.

    rope(x)[d,t] = x[d,t]*cos[d,t] + x[(d+64)%128, t]*sin_sgn[d,t]
    with sin_sgn negative for d < 64 (matches -x2 in the reference).
    """
    j = np.arange(HD // 2, dtype=np.float64)
    inv_freq = ROPE_BASE ** (-2.0 * j / HD)
    t = np.arange(s, dtype=np.float64)
    phase = np.concatenate([inv_freq, inv_freq])[:, None] * t[None, :]
    cos = np.cos(phase).astype(np.float32)
    sin = np.sin(phase).astype(np.float32)
    sin[: HD // 2] *= -1.0
    return np.ascontiguousarray(cos), np.ascontiguousarray(sin)


def build_program(s=S, dm=DM):
    """Build the per-core Bass program. s, dm scaled down for sim tests."""
    W = s // 4                      # span width (512 full-size)
    NSP = 4
    NKC = dm // P                   # contraction chunks (32)
    KBS = W // P                    # k-blocks per span step (4)
    NTB = s // P                    # t-blocks (16)
    dq, dkv = DQ, DKV
    NH = HEADS_PER_CORE
    NDMB = dm // P

    nc = bacc.Bacc(None, target_bir_lowering=False, debug=True)

    # pre-tiled inputs (see _prep_inputs for layouts)
    xt_q = nc.dram_tensor("xt_q", [NSP * dm, W], F32R, kind="ExternalInput")
    xt_k = nc.dram_tensor("xt_k", [2 * dm, 2 * W], F32R, kind="ExternalInput")
    xt_v = nc.dram_tensor("xt_v", [2 * dm, 2 * W], F32R, kind="ExternalInput")
    wq = nc.dram_tensor("wq", [dm, dq], F32R, kind="ExternalInput")
    wk = nc.dram_tensor("wk", [dm, dkv], F32R, kind="ExternalInput")
    wv = nc.dram_tensor("wv", [dm, dkv], F32R, kind="ExternalInput")
    wo = nc.dram_tensor("wo", [NDMB * P, NH * P], F32R, kind="ExternalInput")
    cos_d = nc.dram_tensor("cos_t", [P, s], F32, kind="ExternalInput")
    sin_d = nc.dram_tensor("sin_t", [P, s], F32, kind="ExternalInput")
    mask_d = nc.dram_tensor("maskneg", [P, KBS * W], F32, kind="ExternalInput")
    ones_d = nc.dram_tensor("ones", [P, 1], F32R, kind="ExternalInput")
    ident_d = nc.dram_tensor("ident", [P, P], F32R, kind="ExternalInput")

    out_s = nc.dram_tensor("out_s", [NSP * dm, W], F32, kind="ExternalOutput")
    kht_out = nc.dram_tensor("kht_out", [dkv, s], F32, kind="ExternalOutput")
    vh_out = nc.dram_tensor("vh_out", [KV_PER_CORE * s, HD], F32,
                            kind="ExternalOutput")

    qt_scr = [nc.dram_tensor(f"qt_scr{i}", [dq, W], F32R) for i in range(NSP)]

    with tile.TileContext(nc) as tc:
        with tc.tile_pool(name="persist", bufs=1) as persist:
            kht_sb = persist.tile([P, KV_PER_CORE, s], F32R, name="kht_sb")
            vh_sb = persist.tile([P, NTB, dkv], F32R, name="vh_sb")
            cos_sb = persist.tile([P, s], F32, name="cos_sb")
            sin_sb = persist.tile([P, s], F32, name="sin_sb")
            ones_sb = persist.tile([P, 1], F32R, name="ones_sb")
            ident_sb = persist.tile([P, P], F32R, name="ident_sb")
            nc.scalar.dma_start(out=cos_sb, in_=cos_d[:, :])
            nc.scalar.dma_start(out=sin_sb, in_=sin_d[:, :])
            nc.scalar.dma_start(out=ones_sb, in_=ones_d[:, :])
            nc.scalar.dma_start(out=ident_sb, in_=ident_d[:, :])

            def rope_evict(pool, psum_t, t0, w, f32r_out, f32_out=None):
                """RoPE a [P, w] psum tile covering positions [t0, t0+w)."""
                cs = cos_sb[:, t0:t0 + w]
                sn = sin_sb[:, t0:t0 + w]
                s_t = pool.tile([P, W], F32, name="rope_s")[:, :w]
                nc.scalar.copy(out=s_t, in_=psum_t)
                rot = pool.tile([P, W], F32, name="rope_rot")[:, :w]
                h = HD // 2
                nc.vector.tensor_copy(out=rot[0:h, :], in_=s_t[h:P, :])
                nc.vector.tensor_copy(out=rot[h:P, :], in_=s_t[0:h, :])
                nc.vector.tensor_tensor(s_t, s_t, cs, MUL)
                nc.vector.tensor_tensor(rot, rot, sn, MUL)
                if f32_out is not None:
                    nc.vector.tensor_tensor(f32_out, s_t, rot, ADD)
                    nc.vector.tensor_copy(out=f32r_out, in_=f32_out)
                else:
                    nc.vector.tensor_tensor(f32r_out, s_t, rot, ADD)

            wq_ctx = tc.tile_pool(name="wqpool", bufs=1)
            wqpool = wq_ctx.__enter__()
            # prefetch the whole Q-projection weight first (biggest input)
            wq_sb = wqpool.tile([P, NKC, dq], F32R, name="wq_sb")
            nc.scalar.dma_start(
                out=wq_sb, in_=wq[:, :].rearrange("(n p) m -> p n m", p=P))

            # ================= Phase A1: K projection + RoPE ==============
            with tc.tile_pool(name="a1w", bufs=3) as a1w, \
                 tc.tile_pool(name="a1x", bufs=3) as a1x, \
                 tc.tile_pool(name="a1e", bufs=2) as a1e, \
                 tc.tile_pool(name="a1p", bufs=1, space="PSUM") as a1p:
                for wave in range(2):          # wave = span pair
                    kpsum = [a1p.tile([P, W], F32, name=f"kpsum{wave}_{i}")
                             for i in range(KV_PER_CORE * 2)]
                    for kc in range(NKC):
                        wk_t = a1w.tile([P, dkv], F32R, name="wk_t")
                        nc.scalar.dma_start(
                            out=wk_t, in_=wk[kc * P:(kc + 1) * P, :])
                        xtk_t = a1x.tile([P, 2 * W], F32R, name="xtk_t")
                        base = wave * dm + kc * P
                        nc.sync.dma_start(out=xtk_t,
                                          in_=xt_k[base:base + P, :])
                        for dk in range(KV_PER_CORE):
                            for s2 in range(2):
                                nc.tensor.matmul(
                                    kpsum[dk * 2 + s2],
                                    lhsT=wk_t[:, dk * HD:(dk + 1) * HD],
                                    rhs=xtk_t[:, s2 * W:(s2 + 1) * W],
                                    start=(kc == 0), stop=(kc == NKC - 1))
                    for dk in range(KV_PER_CORE):
                        for s2 in range(2):
                            sp = wave * 2 + s2
                            kc_f32 = a1e.tile([P, W], F32, name="kc_f32")
                            rope_evict(a1e, kpsum[dk * 2 + s2], sp * W, W,
                                       kht_sb[:, dk, sp * W:(sp + 1) * W],
                                       kc_f32)
                            nc.gpsimd.dma_start(
                                out=kht_out[dk * HD:(dk + 1) * HD,
                                            sp * W:(sp + 1) * W],
                                in_=kc_f32)

            # ===== Phase A2: V projection (transposed) + PE transpose =====
            with tc.tile_pool(name="a2w", bufs=3) as a2w, \
                 tc.tile_pool(name="a2x", bufs=3) as a2x, \
                 tc.tile_pool(name="a2e", bufs=3) as a2e, \
                 tc.tile_pool(name="a2p", bufs=1, space="PSUM") as a2p, \
                 tc.tile_pool(name="a2tp", bufs=2, space="PSUM") as a2tp:
                for wave in range(2):          # wave = span pair (t halves)
                    vtpsum = [a2p.tile([P, W], F32, name=f"vtpsum{i}")
                              for i in range(KV_PER_CORE * 2)]
                    for kc in range(NKC):
                        wv_t = a2w.tile([P, dkv], F32R, name="wv_t")
                        nc.scalar.dma_start(
                            out=wv_t, in_=wv[kc * P:(kc + 1) * P, :])
                        xtv_t = a2x.tile([P, 2 * W], F32R, name="xtv_t")
                        base = wave * dm + kc * P
                        nc.sync.dma_start(out=xtv_t,
                                          in_=xt_v[base:base + P, :])
                        for dvh in range(KV_PER_CORE):
                            for s2 in range(2):
                                nc.tensor.matmul(
                                    vtpsum[dvh * 2 + s2],
                                    lhsT=wv_t[:, dvh * HD:(dvh + 1) * HD],
                                    rhs=xtv_t[:, s2 * W:(s2 + 1) * W],
                                    start=(kc == 0), stop=(kc == NKC - 1))
                    for dvh in range(KV_PER_CORE):
                        for s2 in range(2):
                            sp = wave * 2 + s2
                            # VhT tile [dv 128, t W] -> 4 PE transposes
                            vt_sb = a2e.tile([P, W], F32R, name="vt_sb")
                            nc.scalar.copy(out=vt_sb, in_=vtpsum[dvh * 2 + s2])
                            for j in range(KBS):
                                tb = sp * KBS + j
                                tpsum = a2tp.tile([P, P], F32R, name="tpsum")
                                nc.tensor.transpose(
                                    tpsum, vt_sb[:, j * P:(j + 1) * P],
                                    ident_sb)
                                nc.vector.tensor_copy(
                                    out=vh_sb[:, tb, dvh * HD:(dvh + 1) * HD],
                                    in_=tpsum)
                                vc_f32 = a2e.tile([P, P], F32, name="vc_f32")
                                nc.scalar.copy(out=vc_f32, in_=tpsum)
                                nc.gpsimd.dma_start(
                                    out=vh_out[dvh * s + tb * P:
                                               dvh * s + (tb + 1) * P, :],
                                    in_=vc_f32)

            # ================= Phase A3: Q projection + RoPE -> DRAM ======
            with tc.tile_pool(name="a3x", bufs=4) as a3x, \
                 tc.tile_pool(name="a3e", bufs=2) as a3e, \
                 tc.tile_pool(name="a3p", bufs=1, space="PSUM") as a3p:
                for sp in range(NSP):
                    qpsum = [a3p.tile([P, W], F32, name=f"qpsum{i}")
                             for i in range(NH)]
                    for kc in range(NKC):
                        xtq_t = a3x.tile([P, W], F32R, name="xtq_t")
                        base = sp * dm + kc * P
                        nc.sync.dma_start(out=xtq_t,
                                          in_=xt_q[base:base + P, :])
                        for dqb in range(NH):
                            nc.tensor.matmul(
                                qpsum[dqb],
                                lhsT=wq_sb[:, kc, dqb * HD:(dqb + 1) * HD],
                                rhs=xtq_t,
                                start=(kc == 0), stop=(kc == NKC - 1))
                    for dqb in range(NH):
                        q_f32r = a3e.tile([P, W], F32R, name="q_f32r")
                        rope_evict(a3e, qpsum[dqb], sp * W, W, q_f32r)
                        nc.sync.dma_start(
                            out=qt_scr[sp][dqb * HD:(dqb + 1) * HD, :],
                            in_=q_f32r)

            wq_ctx.__exit__(None, None, None)

            # ====== Phases B (attention) + C (out proj), interleaved ======
            with tc.tile_pool(name="bmask", bufs=1) as bmask, \
                 tc.tile_pool(name="bq", bufs=2) as bq, \
                 tc.tile_pool(name="bo", bufs=2) as bo, \
                 tc.tile_pool(name="bw", bufs=3) as bw, \
                 tc.tile_pool(name="bacc", bufs=2) as baccp, \
                 tc.tile_pool(name="bn", bufs=2) as bn, \
                 tc.tile_pool(name="bsp", bufs=1, space="PSUM") as bsp, \
                 tc.tile_pool(name="bop", bufs=2, space="PSUM") as bop, \
                 tc.tile_pool(name="brp", bufs=1, space="PSUM") as brp, \
                 tc.tile_pool(name="cw", bufs=3) as cw, \
                 tc.tile_pool(name="ce", bufs=3) as ce, \
                 tc.tile_pool(name="cp", bufs=1, space="PSUM") as cp:
                mask_sb = bmask.tile([P, KBS, W], F32, name="mask_sb")
                nc.scalar.dma_start(
                    out=mask_sb,
                    in_=mask_d[:, :].rearrange("p (r w) -> p r w", r=KBS))

                def attention_span(sp):
                    qt_sb = bq.tile([HD, NH, W], F32R, name="qt_sb")
                    nc.sync.dma_start(
                        out=qt_sb,
                        in_=qt_scr[sp][:, :].rearrange("(h p) t -> p h t", p=HD))
                    ot_sb = bo.tile([HD, NH, W], F32R, name="ot_sb")
                    kbmax = (sp + 1) * KBS
                    for h in range(NH):
                        kv = h // GROUP
                        opsum = bop.tile([HD, W], F32, name="opsum")
                        # two independent accumulation chains: DVE + GPSIMD
                        accv = baccp.tile([P, W], F32R, name="accv")
                        accg = (baccp.tile([P, W], F32R, name="accg")
                                if kbmax > 1 else None)
                        for kb0 in range(0, kbmax, 2):
                            nk = min(2, kbmax - kb0)
                            spsum = bsp.tile([P, 2, W], F32,
                                             name="spsum")[:, :nk, :]
                            for i in range(nk):
                                nc.tensor.matmul(
                                    spsum[:, i, :],
                                    lhsT=kht_sb[:, kv,
                                                (kb0 + i) * P:(kb0 + i + 1) * P],
                                    rhs=qt_sb[:, h, :],
                                    start=True, stop=True)
                            tanh_t = bw.tile([P, 2, W], F32,
                                             name="tanh_t")[:, :nk, :]
                            nc.scalar.activation(
                                tanh_t, spsum, AF.Tanh,
                                scale=float(ATTN_MULT / MAX_ATTN))
                            r0 = kb0 - sp * KBS
                            if r0 + nk > 0:
                                rs = max(r0, 0)
                                o = rs - r0
                                nc.vector.tensor_tensor(
                                    tanh_t[:, o:nk, :], tanh_t[:, o:nk, :],
                                    mask_sb[:, rs:rs + nk - o, :], ADD)
                            p_t = bw.tile([P, 2, W], F32R,
                                          name="p_t")[:, :nk, :]
                            nc.scalar.activation(
                                p_t, tanh_t, AF.Exp, scale=float(MAX_ATTN))
                            for i in range(nk):
                                nc.tensor.matmul(
                                    opsum,
                                    lhsT=vh_sb[:, kb0 + i,
                                               kv * HD:(kv + 1) * HD],
                                    rhs=p_t[:, i, :],
                                    start=(kb0 + i == 0),
                                    stop=(kb0 + i == kbmax - 1))
                            for i in range(nk):
                                kb = kb0 + i
                                eng = nc.vector if (kb % 2 == 0) else nc.gpsimd
                                acc = accv if (kb % 2 == 0) else accg
                                if kb <= 1:
                                    eng.tensor_copy(out=acc, in_=p_t[:, i, :])
                                else:
                                    eng.tensor_tensor(
                                        acc, acc, p_t[:, i, :], ADD)
                        rpsumv = brp.tile([1, W], F32, name="rpsumv")
                        nc.tensor.matmul(rpsumv, lhsT=ones_sb,
                                         rhs=accv, start=True, stop=True)
                        if kbmax > 1:
                            rpsumg = brp.tile([1, W], F32, name="rpsumg")
                            nc.tensor.matmul(rpsumg, lhsT=ones_sb,
                                             rhs=accg, start=True, stop=True)
                        rsum = bn.tile([1, W], F32, name="rsum")
                        nc.vector.tensor_copy(out=rsum, in_=rpsumv)
                        if kbmax > 1:
                            nc.vector.tensor_tensor(
                                rsum, rsum, rpsumg, ADD)
                        recip = bn.tile([1, W], F32, name="recip")
                        nc.vector.reciprocal_approx_fast(recip, rsum)
                        rb = bn.tile([P, W], F32, name="rb")
                        nc.gpsimd.partition_broadcast(rb, recip)
                        nc.vector.tensor_tensor(
                            ot_sb[:, h, :], opsum, rb, MUL)
                    return ot_sb

                def outproj_half(half, ot_tiles):
                    for dmb in range(NDMB):
                        wo_t = cw.tile([P, NH, P], F32R, name="wo_t")
                        nc.scalar.dma_start(
                            out=wo_t,
                            in_=wo[dmb * P:(dmb + 1) * P, :].rearrange(
                                "p (h m) -> p h m", h=NH))
                        cpsums = [cp.tile([P, W], F32, name=f"cpsum{s2}")
                                  for s2 in range(2)]
                        # qc outer: stationary wo_t[:, qc, :] reused for both
                        # spans (ldw-opt dedups the second LDWEIGHTS)
                        for qc in range(NH):
                            for s2 in range(2):
                                nc.tensor.matmul(
                                    cpsums[s2],
                                    lhsT=wo_t[:, qc, :],
                                    rhs=ot_tiles[s2][:, qc, :],
                                    start=(qc == 0), stop=(qc == NH - 1))
                        for s2 in range(2):
                            sp = half * 2 + s2
                            cout = ce.tile([P, W], F32, name="cout")
                            nc.vector.tensor_copy(out=cout, in_=cpsums[s2])
                            base = sp * dm + dmb * P
                            nc.gpsimd.dma_start(
                                out=out_s[base:base + P, :], in_=cout)

                ot0 = attention_span(0)
                ot1 = attention_span(1)
                outproj_half(0, [ot0, ot1])
                ot2 = attention_span(2)
                ot3 = attention_span(3)
                outproj_half(1, [ot2, ot3])

    nc.compile()
    return nc


def _prep_inputs(query, key, value, Wq, Wk, Wv, Wo, s, dm):
    """Build the 8 per-core input maps from the full tensors."""
    W = s // 4
    KBS = W // P
    NH = HEADS_PER_CORE
    NDMB = dm // P
    cos, sin = _rope_tables(s)
    maskneg = np.zeros((P, KBS * W), dtype=np.float32)
    for r in range(KBS):
        k_idx = np.arange(P)[:, None] + r * P
        q_idx = np.arange(W)[None, :]
        maskneg[:, r * W:(r + 1) * W] = np.where(
            k_idx <= q_idx, 0.0, MASK_NEG).astype(np.float32)
    ones = np.ones((P, 1), dtype=np.float32)
    ident = np.eye(P, dtype=np.float32)

    xt = []
    for b in range(B):
        xqT = np.ascontiguousarray(query[b].T)    # [dm, s]
        xkT = np.ascontiguousarray(key[b].T)
        xvT = np.ascontiguousarray(value[b].T)
        xq_s = np.ascontiguousarray(
            xqT.reshape(dm, 4, W).transpose(1, 0, 2).reshape(4 * dm, W))
        xk_s = np.ascontiguousarray(
            xkT.reshape(dm, 2, 2 * W).transpose(1, 0, 2).reshape(2 * dm, 2 * W))
        xv_s = np.ascontiguousarray(
            xvT.reshape(dm, 2, 2 * W).transpose(1, 0, 2).reshape(2 * dm, 2 * W))
        xt.append((xq_s, xk_s, xv_s))

    in_maps = []
    for c in range(NCORES):
        b, j = divmod(c, NCORES // B)
        xq_s, xk_s, xv_s = xt[b]
        wo_j = Wo[j * DQ:(j + 1) * DQ, :]         # [DQ, dm]
        wo_pre = np.ascontiguousarray(
            wo_j.reshape(NH, P, NDMB, P).transpose(2, 1, 0, 3).reshape(
                NDMB * P, NH * P))
        in_maps.append({
            "xt_q": xq_s, "xt_k": xk_s, "xt_v": xv_s,
            "wq": np.ascontiguousarray(Wq[:, j * DQ:(j + 1) * DQ]),
            "wk": np.ascontiguousarray(Wk[:, j * DKV:(j + 1) * DKV]),
            "wv": np.ascontiguousarray(Wv[:, j * DKV:(j + 1) * DKV]),
            "wo": wo_pre,
            "cos_t": cos, "sin_t": sin, "maskneg": maskneg, "ones": ones,
            "ident": ident,
        })
    return in_maps


def kernel(query, key, value, mask, Wq, Wk, Wv, Wo, trace=False):
    """Full-size entry point: full inputs in, full outputs out."""
    _patch_ldw_opt()
    query = np.asarray(query, dtype=np.float32)
    key = np.asarray(key, dtype=np.float32)
    value = np.asarray(value, dtype=np.float32)
    Wq = np.asarray(Wq, dtype=np.float32)
    Wk = np.asarray(Wk, dtype=np.float32)
    Wv = np.asarray(Wv, dtype=np.float32)
    Wo = np.asarray(Wo, dtype=np.float32)
    # mask is causal by construction (tril); the kernel exploits it directly.

    nc = build_program(S, DM)
    in_maps = _prep_inputs(query, key, value, Wq, Wk, Wv, Wo, S, DM)
    res = run_bass_kernel_spmd(nc, in_maps, list(range(NCORES)), trace=trace)

    out = np.zeros((B, S, DM), dtype=np.float32)
    cache_k = np.zeros((B, NKV, S, HD), dtype=np.float32)
    cache_v = np.zeros((B, NKV, S, HD), dtype=np.float32)
    W = S // 4
    JP = NCORES // B
    for c in range(NCORES):
        b, j = divmod(c, JP)
        r = res.results[c]
        o = r["out_s"].reshape(4, DM, W)          # [sp, dm, W]
        for sp in range(4):
            out[b, sp * W:(sp + 1) * W, :] += o[sp].T
        kht = r["kht_out"]              # [DKV, S]
        vh = r["vh_out"]                # [KV_PER_CORE*S, HD]
        for i in range(KV_PER_CORE):
            cache_k[b, KV_PER_CORE * j + i] = kht[i * HD:(i + 1) * HD, :].T
            cache_v[b, KV_PER_CORE * j + i] = vh[i * S:(i + 1) * S, :]
    if trace:
        kernel._last_exec_time_ns = res.exec_time_ns
    return out, cache_k, cache_v


# revision 21
# speedup vs baseline: 1.3188x; 1.3188x over previous
"""Trainium2 Bass kernel for CachingMultiHeadAttention (GQA + RoPE + softcap).

Reference semantics (B=2, S=2048, D=4096, 32 q-heads, 8 kv-heads, hd=128):
    qh = rope(x_q @ Wq); kh = rope(x_k @ Wk); vh = x_v @ Wv
    logits = softcap_30(qh kh^T / sqrt(128)) causal-masked
    out = softmax(logits) vh @ Wo ; also returns cache_k, cache_v

Sharding: 8 cores = (2 batches) x (4 kv-head pairs). Each core computes its
batch's projections for its 2 kv-heads / 8 q-heads against full D_MODEL,
attention, and a partial out^T (summed on host over the 4 cores per batch).

All device matmuls run in float32r (fast fp32 mode, 1 cyc/row at N>=256).
Activations are fed pre-transposed and pre-tiled from the host so every
matmul contracts over the partition dim and every DMA is a large mostly-
contiguous transfer. DMA traffic is spread over three queues (sync/scalar
HWDGE + gpsimd SWDGE). Walrus ldw-opt is enabled (see _patch_ldw_opt) so
consecutive matmuls sharing a stationary skip redundant LDWEIGHTS.

Pipeline:
  A0 wq prefetch (one 16.8MB DMA issued first; wk/wv are streamed per-chunk)
  A1 K proj (+RoPE) -> kht_sb resident      [2 waves, ping-pong PSUM]
  A2 V proj (transposed) + PE-transpose -> vh_sb resident [2 waves]
  A3 Q proj (+RoPE) -> qt_scr[span] DRAM
  B  attention per span -> ot_sb (SBUF)     [softcap=ACT tanh/exp pairs,
                                             additive -100 mask pre-exp,
                                             rowsum = ones-matmul over
                                             DVE+GPSIMD-accumulated P,
                                             approx-recip + partition_bcast]
  C  out proj, 2 half phases (spans 01/23), interleaved with B
"""
import sys
sys.path.insert(0, "/opt/trn_rl_repo")

import numpy as np

import concourse.bass as bass
from concourse import bacc
import concourse.mybir as mybir
import concourse.tile as tile
import concourse.alu_op_type as alu
import concourse.bass_utils as bass_utils
from concourse.bass_utils import run_bass_kernel_spmd

F32 = mybir.dt.float32
F32R = mybir.dt.float32r
AF = mybir.ActivationFunctionType
MUL = alu.AluOpType.mult
ADD = alu.AluOpType.add

# Problem constants
B, S, DM = 2, 2048, 4096
NQ, NKV, HD = 32, 8, 128
GROUP = NQ // NKV
ATTN_MULT = 1.0 / np.sqrt(128.0)
MAX_ATTN = 30.0
ROPE_BASE = 10000.0
MASK_NEG = -100.0            # additive pre-exp mask value (exp(30*-100) == 0)

P = 128
NCORES = 8
HEADS_PER_CORE = NQ // (NCORES // B)      # 8
KV_PER_CORE = NKV // (NCORES // B)        # 2
DQ = HEADS_PER_CORE * HD                  # 1024
DKV = KV_PER_CORE * HD                    # 256

_LDW_PATCHED = False


def _patch_ldw_opt():
    """Enable walrus's LDWEIGHTS dedup pass (hardcoded off in bass_utils)."""
    global _LDW_PATCHED
    if _LDW_PATCHED:
        return
    _LDW_PATCHED = True
    orig = bass_utils.run_command

    def run_command_ldwopt(argv, **kw):
        argv = ['--enable-ldw-opt=true' if a == '--enable-ldw-opt=false'
                else a for a in argv]
        return orig(argv, **kw)

    bass_utils.run_command = run_command_ldwopt


def _rope_tables(s):
    """cos table and sign-baked sin table, [HD, s] f32.

    rope(x)[d,t] = x[d,t]*cos[d,t] + x[(d+64)%128, t]*sin_sgn[d,t]
    with sin_sgn negative for d < 64 (matches -x2 in the reference).
    """
    j = np.arange(HD // 2, dtype=np.float64)
    inv_freq = ROPE_BASE ** (-2.0 * j / HD)
    t = np.arange(s, dtype=np.float64)
    phase = np.concatenate([inv_freq, inv_freq])[:, None] * t[None, :]
    cos = np.cos(phase).astype(np.float32)
    sin = np.sin(phase).astype(np.float32)
    sin[: HD // 2] *= -1.0
    return np.ascontiguousarray(cos), np.ascontiguousarray(sin)


def build_program(s=S, dm=DM):
    """Build the per-core Bass program. s, dm scaled down for sim tests."""
    W = s // 4                      # span width (512 full-size)
    NSP = 4
    NKC = dm // P                   # contraction chunks (32)
    KBS = W // P                    # k-blocks per span step (4)
    NTB = s // P                    # t-blocks (16)
    dq, dkv = DQ, DKV
    NH = HEADS_PER_CORE
    NDMB = dm // P

    nc = bacc.Bacc(None, target_bir_lowering=False, debug=True)

    # pre-tiled inputs (see _prep_inputs for layouts)
    xt_q = nc.dram_tensor("xt_q", [NSP * dm, W], F32R, kind="ExternalInput")
    xt_k = nc.dram_tensor("xt_k", [2 * dm, 2 * W], F32R, kind="ExternalInput")
    xt_v = nc.dram_tensor("xt_v", [2 * dm, 2 * W], F32R, kind="ExternalInput")
    wq = nc.dram_tensor("wq", [dm, dq], F32R, kind="ExternalInput")
    wk = nc.dram_tensor("wk", [dm, dkv], F32R, kind="ExternalInput")
    wv = nc.dram_tensor("wv", [dm, dkv], F32R, kind="ExternalInput")
    wo = nc.dram_tensor("wo", [NDMB * P, NH * P], F32R, kind="ExternalInput")
    cos_d = nc.dram_tensor("cos_t", [P, s], F32, kind="ExternalInput")
    sin_d = nc.dram_tensor("sin_t", [P, s], F32, kind="ExternalInput")
    mask_d = nc.dram_tensor("maskneg", [P, KBS * W], F32, kind="ExternalInput")
    ones_d = nc.dram_tensor("ones", [P, 1], F32R, kind="ExternalInput")
    ident_d = nc.dram_tensor("ident", [P, P], F32R, kind="ExternalInput")

    out_s = nc.dram_tensor("out_s", [NSP * dm, W], F32, kind="ExternalOutput")
    kht_out = nc.dram_tensor("kht_out", [dkv, s], F32, kind="ExternalOutput")
    vh_out = nc.dram_tensor("vh_out", [KV_PER_CORE * s, HD], F32,
                            kind="ExternalOutput")

    qt_scr = [nc.dram_tensor(f"qt_scr{i}", [dq, W], F32R) for i in range(NSP)]

    with tile.TileContext(nc) as tc:
        with tc.tile_pool(name="persist", bufs=1) as persist:
            kht_sb = persist.tile([P, KV_PER_CORE, s], F32R, name="kht_sb")
            vh_sb = persist.tile([P, NTB, dkv], F32R, name="vh_sb")
            cos_sb = persist.tile([P, s], F32, name="cos_sb")
            sin_sb = persist.tile([P, s], F32, name="sin_sb")
            ones_sb = persist.tile([P, 1], F32R, name="ones_sb")
            ident_sb = persist.tile([P, P], F32R, name="ident_sb")
            nc.scalar.dma_start(out=cos_sb, in_=cos_d[:, :])
            nc.scalar.dma_start(out=sin_sb, in_=sin_d[:, :])
            nc.scalar.dma_start(out=ones_sb, in_=ones_d[:, :])
            nc.scalar.dma_start(out=ident_sb, in_=ident_d[:, :])

            def rope_evict(pool, psum_t, t0, w, f32r_out, f32_out=None):
                """RoPE a [P, w] psum tile covering positions [t0, t0+w)."""
                cs = cos_sb[:, t0:t0 + w]
                sn = sin_sb[:, t0:t0 + w]
                s_t = pool.tile([P, W], F32, name="rope_s")[:, :w]
                nc.scalar.copy(out=s_t, in_=psum_t)
                rot = pool.tile([P, W], F32, name="rope_rot")[:, :w]
                h = HD // 2
                nc.vector.tensor_copy(out=rot[0:h, :], in_=s_t[h:P, :])
                nc.vector.tensor_copy(out=rot[h:P, :], in_=s_t[0:h, :])
                nc.vector.tensor_tensor(s_t, s_t, cs, MUL)
                nc.vector.tensor_tensor(rot, rot, sn, MUL)
                if f32_out is not None:
                    nc.vector.tensor_tensor(f32_out, s_t, rot, ADD)
                    nc.vector.tensor_copy(out=f32r_out, in_=f32_out)
                else:
                    nc.vector.tensor_tensor(f32r_out, s_t, rot, ADD)

            wq_ctx = tc.tile_pool(name="wqpool", bufs=1)
            wqpool = wq_ctx.__enter__()
            # prefetch the whole Q-projection weight first (biggest input)
            wq_sb = wqpool.tile([P, NKC, dq], F32R, name="wq_sb")
            nc.scalar.dma_start(
                out=wq_sb, in_=wq[:, :].rearrange("(n p) m -> p n m", p=P))

            # ================= Phase A1: K projection + RoPE ==============
            with tc.tile_pool(name="a1w", bufs=3) as a1w, \
                 tc.tile_pool(name="a1x", bufs=3) as a1x, \
                 tc.tile_pool(name="a1e", bufs=2) as a1e, \
                 tc.tile_pool(name="a1p", bufs=1, space="PSUM") as a1p:
                for wave in range(2):          # wave = span pair
                    kpsum = [a1p.tile([P, W], F32, name=f"kpsum{wave}_{i}")
                             for i in range(KV_PER_CORE * 2)]
                    for kc in range(NKC):
                        wk_t = a1w.tile([P, dkv], F32R, name="wk_t")
                        nc.scalar.dma_start(
                            out=wk_t, in_=wk[kc * P:(kc + 1) * P, :])
                        xtk_t = a1x.tile([P, 2 * W], F32R, name="xtk_t")
                        base = wave * dm + kc * P
                        nc.sync.dma_start(out=xtk_t,
                                          in_=xt_k[base:base + P, :])
                        for dk in range(KV_PER_CORE):
                            for s2 in range(2):
                                nc.tensor.matmul(
                                    kpsum[dk * 2 + s2],
                                    lhsT=wk_t[:, dk * HD:(dk + 1) * HD],
                                    rhs=xtk_t[:, s2 * W:(s2 + 1) * W],
                                    start=(kc == 0), stop=(kc == NKC - 1))
                    for dk in range(KV_PER_CORE):
                        for s2 in range(2):
                            sp = wave * 2 + s2
                            kc_f32 = a1e.tile([P, W], F32, name="kc_f32")
                            rope_evict(a1e, kpsum[dk * 2 + s2], sp * W, W,
                                       kht_sb[:, dk, sp * W:(sp + 1) * W],
                                       kc_f32)
                            nc.gpsimd.dma_start(
                                out=kht_out[dk * HD:(dk + 1) * HD,
                                            sp * W:(sp + 1) * W],
                                in_=kc_f32)

            # ===== Phase A2: V projection (transposed) + PE transpose =====
            with tc.tile_pool(name="a2w", bufs=3) as a2w, \
                 tc.tile_pool(name="a2x", bufs=3) as a2x, \
                 tc.tile_pool(name="a2e", bufs=3) as a2e, \
                 tc.tile_pool(name="a2p", bufs=1, space="PSUM") as a2p, \
                 tc.tile_pool(name="a2tp", bufs=2, space="PSUM") as a2tp:
                for wave in range(2):          # wave = span pair (t halves)
                    vtpsum = [a2p.tile([P, W], F32, name=f"vtpsum{i}")
                              for i in range(KV_PER_CORE * 2)]
                    for kc in range(NKC):
                        wv_t = a2w.tile([P, dkv], F32R, name="wv_t")
                        nc.scalar.dma_start(
                            out=wv_t, in_=wv[kc * P:(kc + 1) * P, :])
                        xtv_t = a2x.tile([P, 2 * W], F32R, name="xtv_t")
                        base = wave * dm + kc * P
                        nc.sync.dma_start(out=xtv_t,
                                          in_=xt_v[base:base + P, :])
                        for dvh in range(KV_PER_CORE):
                            for s2 in range(2):
                                nc.tensor.matmul(
                                    vtpsum[dvh * 2 + s2],
                                    lhsT=wv_t[:, dvh * HD:(dvh + 1) * HD],
                                    rhs=xtv_t[:, s2 * W:(s2 + 1) * W],
                                    start=(kc == 0), stop=(kc == NKC - 1))
                    for dvh in range(KV_PER_CORE):
                        for s2 in range(2):
                            sp = wave * 2 + s2
                            # VhT tile [dv 128, t W] -> 4 PE transposes
                            vt_sb = a2e.tile([P, W], F32R, name="vt_sb")
                            nc.scalar.copy(out=vt_sb, in_=vtpsum[dvh * 2 + s2])
                            for j in range(KBS):
                                tb = sp * KBS + j
                                tpsum = a2tp.tile([P, P], F32R, name="tpsum")
                                nc.tensor.transpose(
                                    tpsum, vt_sb[:, j * P:(j + 1) * P],
                                    ident_sb)
                                nc.vector.tensor_copy(
                                    out=vh_sb[:, tb, dvh * HD:(dvh + 1) * HD],
                                    in_=tpsum)
                                vc_f32 = a2e.tile([P, P], F32, name="vc_f32")
                                nc.scalar.copy(out=vc_f32, in_=tpsum)
                                nc.gpsimd.dma_start(
                                    out=vh_out[dvh * s + tb * P:
                                               dvh * s + (tb + 1) * P, :],
                                    in_=vc_f32)

            # ================= Phase A3: Q projection + RoPE -> DRAM ======
            with tc.tile_pool(name="a3x", bufs=4) as a3x, \
                 tc.tile_pool(name="a3e", bufs=2) as a3e, \
                 tc.tile_pool(name="a3p", bufs=1, space="PSUM") as a3p:
                for sp in range(NSP):
                    qpsum = [a3p.tile([P, W], F32, name=f"qpsum{i}")
                             for i in range(NH)]
                    for kc in range(NKC):
                        xtq_t = a3x.tile([P, W], F32R, name="xtq_t")
                        base = sp * dm + kc * P
                        nc.sync.dma_start(out=xtq_t,
                                          in_=xt_q[base:base + P, :])
                        for dqb in range(NH):
                            nc.tensor.matmul(
                                qpsum[dqb],
                                lhsT=wq_sb[:, kc, dqb * HD:(dqb + 1) * HD],
                                rhs=xtq_t,
                                start=(kc == 0), stop=(kc == NKC - 1))
                    for dqb in range(NH):
                        q_f32r = a3e.tile([P, W], F32R, name="q_f32r")
                        rope_evict(a3e, qpsum[dqb], sp * W, W, q_f32r)
                        nc.sync.dma_start(
                            out=qt_scr[sp][dqb * HD:(dqb + 1) * HD, :],
                            in_=q_f32r)

            wq_ctx.__exit__(None, None, None)

            # ====== Phases B (attention) + C (out proj), interleaved ======
            with tc.tile_pool(name="bmask", bufs=1) as bmask, \
                 tc.tile_pool(name="bq", bufs=2) as bq, \
                 tc.tile_pool(name="bo", bufs=4) as bo, \
                 tc.tile_pool(name="bw", bufs=2) as bw, \
                 tc.tile_pool(name="bacc", bufs=2) as baccp, \
                 tc.tile_pool(name="bn", bufs=2) as bn, \
                 tc.tile_pool(name="bsp", bufs=2, space="PSUM") as bsp, \
                 tc.tile_pool(name="bop", bufs=1, space="PSUM") as bop, \
                 tc.tile_pool(name="brp", bufs=1, space="PSUM") as brp, \
                 tc.tile_pool(name="cw", bufs=3) as cw, \
                 tc.tile_pool(name="ce", bufs=3) as ce, \
                 tc.tile_pool(name="cp", bufs=1, space="PSUM") as cp:
                mask_sb = bmask.tile([P, KBS, W], F32, name="mask_sb")
                nc.scalar.dma_start(
                    out=mask_sb,
                    in_=mask_d[:, :].rearrange("p (r w) -> p r w", r=KBS))

                def emit_qk(sp, qt_sb, kv, h, kb0, nk):
                    """QK matmuls for one kb pair -> fresh spsum tile."""
                    spsum = bsp.tile([P, 2, W], F32, name="spsum")[:, :nk, :]
                    for i in range(nk):
                        nc.tensor.matmul(
                            spsum[:, i, :],
                            lhsT=kht_sb[:, kv, (kb0 + i) * P:(kb0 + i + 1) * P],
                            rhs=qt_sb[:, h, :],
                            start=True, stop=True)
                    return spsum

                def attention_span(sp, filler=None):
                    qt_sb = bq.tile([HD, NH, W], F32R, name="qt_sb")
                    nc.sync.dma_start(
                        out=qt_sb,
                        in_=qt_scr[sp][:, :].rearrange("(h p) t -> p h t", p=HD))
                    ot_sb = bo.tile([HD, NH, W], F32R, name="ot_sb")
                    kbmax = (sp + 1) * KBS
                    pairs = [(kb0, min(2, kbmax - kb0))
                             for kb0 in range(0, kbmax, 2)]
                    for h in range(NH):
                        kv = h // GROUP
                        opsum = bop.tile([HD, W], F32, name="opsum")
                        accv = baccp.tile([P, W], F32R, name="accv")
                        # software pipeline: QK one pair ahead of PV
                        spsums = [emit_qk(sp, qt_sb, kv, h, *pairs[0])]
                        for pr, (kb0, nk) in enumerate(pairs):
                            if pr + 1 < len(pairs):
                                spsums.append(
                                    emit_qk(sp, qt_sb, kv, h, *pairs[pr + 1]))
                            spsum = spsums[pr]
                            tanh_t = bw.tile([P, 2, W], F32,
                                             name="tanh_t")[:, :nk, :]
                            nc.scalar.activation(
                                tanh_t, spsum, AF.Tanh,
                                scale=float(ATTN_MULT / MAX_ATTN))
                            r0 = kb0 - sp * KBS
                            if r0 + nk > 0:
                                rs = max(r0, 0)
                                o = rs - r0
                                nc.vector.tensor_tensor(
                                    tanh_t[:, o:nk, :], tanh_t[:, o:nk, :],
                                    mask_sb[:, rs:rs + nk - o, :], ADD)
                            p_t = bw.tile([P, 2, W], F32R,
                                          name="p_t")[:, :nk, :]
                            nc.scalar.activation(
                                p_t, tanh_t, AF.Exp, scale=float(MAX_ATTN))
                            for i in range(nk):
                                nc.tensor.matmul(
                                    opsum,
                                    lhsT=vh_sb[:, kb0 + i,
                                               kv * HD:(kv + 1) * HD],
                                    rhs=p_t[:, i, :],
                                    start=(kb0 + i == 0),
                                    stop=(kb0 + i == kbmax - 1))
                            for i in range(nk):
                                if kb0 + i == 0:
                                    nc.vector.tensor_copy(
                                        out=accv, in_=p_t[:, i, :])
                                else:
                                    nc.vector.tensor_tensor(
                                        accv, accv, p_t[:, i, :], ADD)
                        rpsumv = brp.tile([1, W], F32, name="rpsumv")
                        nc.tensor.matmul(rpsumv, lhsT=ones_sb,
                                         rhs=accv, start=True, stop=True)
                        rsum = bn.tile([1, W], F32, name="rsum")
                        nc.vector.tensor_copy(out=rsum, in_=rpsumv)
                        recip = bn.tile([1, W], F32, name="recip")
                        nc.vector.reciprocal_approx_fast(recip, rsum)
                        rb = bn.tile([P, W], F32, name="rb")
                        nc.gpsimd.partition_broadcast(rb, recip)
                        nc.vector.tensor_tensor(
                            ot_sb[:, h, :], opsum, rb, MUL)
                        if filler is not None:
                            filler()
                    return ot_sb

                def outproj_dmb(half, dmb, ot_tiles):
                    """One out-projection unit: [128 dm rows x 2 spans]."""
                    wo_t = cw.tile([P, NH, P], F32R, name="wo_t")
                    nc.sync.dma_start(
                        out=wo_t,
                        in_=wo[dmb * P:(dmb + 1) * P, :].rearrange(
                            "p (h m) -> p h m", h=NH))
                    cpsums = [cp.tile([P, W], F32, name=f"cpsum{s2}")
                              for s2 in range(2)]
                    # qc outer: stationary wo_t[:, qc, :] reused for both
                    # spans (ldw-opt dedups the second LDWEIGHTS)
                    for qc in range(NH):
                        for s2 in range(2):
                            nc.tensor.matmul(
                                cpsums[s2],
                                lhsT=wo_t[:, qc, :],
                                rhs=ot_tiles[s2][:, qc, :],
                                start=(qc == 0), stop=(qc == NH - 1))
                    for s2 in range(2):
                        sp = half * 2 + s2
                        cout = ce.tile([P, W], F32, name="cout")
                        nc.vector.tensor_copy(out=cout, in_=cpsums[s2])
                        base = sp * dm + dmb * P
                        nc.gpsimd.dma_start(
                            out=out_s[base:base + P, :], in_=cout)

                ot0 = attention_span(0)
                ot1 = attention_span(1)
                c0_left = list(range(NDMB))

                def c0_filler():
                    for _ in range(2):
                        if c0_left:
                            outproj_dmb(0, c0_left.pop(0), [ot0, ot1])

                ot2 = attention_span(2, c0_filler)
                ot3 = attention_span(3, c0_filler)
                while c0_left:
                    outproj_dmb(0, c0_left.pop(0), [ot0, ot1])
                for dmb in range(NDMB):
                    outproj_dmb(1, dmb, [ot2, ot3])

    nc.compile()
    return nc


def _prep_inputs(query, key, value, Wq, Wk, Wv, Wo, s, dm):
    """Build the 8 per-core input maps from the full tensors."""
    W = s // 4
    KBS = W // P
    NH = HEADS_PER_CORE
    NDMB = dm // P
    cos, sin = _rope_tables(s)
    maskneg = np.zeros((P, KBS * W), dtype=np.float32)
    for r in range(KBS):
        k_idx = np.arange(P)[:, None] + r * P
        q_idx = np.arange(W)[None, :]
        maskneg[:, r * W:(r + 1) * W] = np.where(
            k_idx <= q_idx, 0.0, MASK_NEG).astype(np.float32)
    ones = np.ones((P, 1), dtype=np.float32)
    ident = np.eye(P, dtype=np.float32)

    xt = []
    for b in range(B):
        xqT = np.ascontiguousarray(query[b].T)    # [dm, s]
        xkT = np.ascontiguousarray(key[b].T)
        xvT = np.ascontiguousarray(value[b].T)
        xq_s = np.ascontiguousarray(
            xqT.reshape(dm, 4, W).transpose(1, 0, 2).reshape(4 * dm, W))
        xk_s = np.ascontiguousarray(
            xkT.reshape(dm, 2, 2 * W).transpose(1, 0, 2).reshape(2 * dm, 2 * W))
        xv_s = np.ascontiguousarray(
            xvT.reshape(dm, 2, 2 * W).transpose(1, 0, 2).reshape(2 * dm, 2 * W))
        xt.append((xq_s, xk_s, xv_s))

    in_maps = []
    for c in range(NCORES):
        b, j = divmod(c, NCORES // B)
        xq_s, xk_s, xv_s = xt[b]
        wo_j = Wo[j * DQ:(j + 1) * DQ, :]         # [DQ, dm]
        wo_pre = np.ascontiguousarray(
            wo_j.reshape(NH, P, NDMB, P).transpose(2, 1, 0, 3).reshape(
                NDMB * P, NH * P))
        in_maps.append({
            "xt_q": xq_s, "xt_k": xk_s, "xt_v": xv_s,
            "wq": np.ascontiguousarray(Wq[:, j * DQ:(j + 1) * DQ]),
            "wk": np.ascontiguousarray(Wk[:, j * DKV:(j + 1) * DKV]),
            "wv": np.ascontiguousarray(Wv[:, j * DKV:(j + 1) * DKV]),
            "wo": wo_pre,
            "cos_t": cos, "sin_t": sin, "maskneg": maskneg, "ones": ones,
            "ident": ident,
        })
    return in_maps


def kernel(query, key, value, mask, Wq, Wk, Wv, Wo, trace=False):
    """Full-size entry point: full inputs in, full outputs out."""
    _patch_ldw_opt()
    query = np.asarray(query, dtype=np.float32)
    key = np.asarray(key, dtype=np.float32)
    value = np.asarray(value, dtype=np.float32)
    Wq = np.asarray(Wq, dtype=np.float32)
    Wk = np.asarray(Wk, dtype=np.float32)
    Wv = np.asarray(Wv, dtype=np.float32)
    Wo = np.asarray(Wo, dtype=np.float32)
    # mask is causal by construction (tril); the kernel exploits it directly.

    nc = build_program(S, DM)
    in_maps = _prep_inputs(query, key, value, Wq, Wk, Wv, Wo, S, DM)
    res = run_bass_kernel_spmd(nc, in_maps, list(range(NCORES)), trace=trace)

    out = np.zeros((B, S, DM), dtype=np.float32)
    cache_k = np.zeros((B, NKV, S, HD), dtype=np.float32)
    cache_v = np.zeros((B, NKV, S, HD), dtype=np.float32)
    W = S // 4
    JP = NCORES // B
    for c in range(NCORES):
        b, j = divmod(c, JP)
        r = res.results[c]
        o = r["out_s"].reshape(4, DM, W)          # [sp, dm, W]
        for sp in range(4):
            out[b, sp * W:(sp + 1) * W, :] += o[sp].T
        kht = r["kht_out"]              # [DKV, S]
        vh = r["vh_out"]                # [KV_PER_CORE*S, HD]
        for i in range(KV_PER_CORE):
            cache_k[b, KV_PER_CORE * j + i] = kht[i * HD:(i + 1) * HD, :].T
            cache_v[b, KV_PER_CORE * j + i] = vh[i * S:(i + 1) * S, :]
    if trace:
        kernel._last_exec_time_ns = res.exec_time_ns
    return out, cache_k, cache_v
